# revision 47
# baseline (speedup 1.0000x reference)
"""Trainium2 Bass kernel for nn_DGL_GCN (3-layer hetero GCN + MLP head).

Math (reference): 3x hetero layers
    h' = relu( sum_e segment_mean_e( h @ W_e.T + b_e ) )
then z = relu(fc1_w @ h3.flatten() + fc1_b); out = sigmoid(fc2_w @ z + fc2_b).

The per-etype mean aggregation over edges is algebraically
    A_e @ (h @ W_e.T) + ind_e (x) b_e
with A_e[dst, src] = multiplicity(src->dst) / max(deg(dst),1) and
ind_e[dst] = deg(dst) > 0. A_e is a fixed 4096x4096 matrix per etype, so
each layer is dense matmuls on the PE array.

Sharding over 8 cores: destination-node shards (512 dst nodes per core,
all 8 etypes on-core -> all cross-etype sums happen in fp32 PSUM, no
AllReduce needed; one small AllGather of h per layer). fc1 is column-
sharded over the flattened node*hidden dim (each core's own h3 shard is
exactly its fc1 column slice); partial z vectors are AllGather+summed.

All heavy matmuls run fp8 x fp8 with perf_mode=DoubleRow (K=256 per
pass, 2 MACs/cell/cycle) accumulating in fp32 PSUM. fc1's thin-M
(M=1) matmuls are packed 4-wide across PE column groups via
tile_position so four run concurrently.
"""

import numpy as np
import ml_dtypes

N_OBJ = 4096
F_IN = 256
H = 256
C = 128
NE = 8
NCORES = 8
SHARD = N_OBJ // NCORES          # 512 dst nodes per core
NT = N_OBJ // 128                # 32 node tiles
NP = NT // 2                     # 16 k-tile pairs (DoubleRow)
ABLK = 4                         # kt-pairs batched per A DMA
FCB = 16                         # fc1 k-tiles batched per DMA
FCK = (SHARD * H) // 128         # 1024 fc1 k-tiles per core
FC_PREFETCH_BUFS = 26            # fc1 tiles resident during layers

BF16 = ml_dtypes.bfloat16
FP8 = ml_dtypes.float8_e4m3
FC1_SCALE = 8192.0  # fc1_w ~N(0, 0.002) is subnormal in e4m3; pre-scale
H_SCALE = 16.0      # hidden state h kept as 16*h in fp8
W_SCALE = 4.0       # per-etype W kept as 4*W.T in fp8 (keeps X < 240)
A_SCALE = 4.0       # adjacency kept as 4*A in fp8
OUT_SCALE = H_SCALE * W_SCALE * A_SCALE  # PSUM domain of a layer output

_BASS_CACHE = {}


def _split_drain_waits(nc, max_waits=1):
    # This walrus build accepts only one sync-wait command on an InstDrain;
    # Tile's tail drain waits on every active proc lane. Split into a chain
    # of single-wait drains.
    import copy
    import concourse.mybir as mybir

    for f in nc.m.functions:
        for bb in f.blocks:
            new_list = []
            for ins in bb.instructions:
                si = ins.sync_info
                if (
                    isinstance(ins, mybir.InstDrain)
                    and si is not None
                    and si.on_wait
                    and len(si.on_wait) > max_waits
                ):
                    waits = list(si.on_wait)
                    updates = list(si.on_update or [])
                    for i, w in enumerate(waits[:-1]):
                        d = copy.deepcopy(ins)
                        d.name = f"{ins.name}-sw{i}"
                        dsi = d.sync_info
                        dsi.on_wait = [w]
                        dsi.on_update = []
                        d.sync_info = dsi
                        new_list.append(d)
                        nc.inst_map[d.name] = d
                    si.on_wait = [waits[-1]]
                    si.on_update = updates
                    ins.sync_info = si
                new_list.append(ins)
            bb.instructions[:] = new_list


def _build_bass(n_layers=3):
    import concourse.bass as bass  # noqa: F401
    import concourse.tile as tile
    import concourse.mybir as mybir
    from concourse import bacc

    f32 = mybir.dt.float32
    bf16 = mybir.dt.bfloat16
    fp8 = mybir.dt.float8e4
    AF = mybir.ActivationFunctionType
    DR = mybir.MatmulPerfMode.DoubleRow

    nc = bacc.Bacc(
        "TRN2", target_bir_lowering=False, debug=False, num_devices=NCORES
    )

    # ---- I/O (per-core values supplied via in_maps) ----
    G0 = nc.dram_tensor("g0", [128, 2, N_OBJ], fp8, kind="ExternalInput")
    ATP = nc.dram_tensor(
        "atp", [NE, NP // ABLK, 128, ABLK * 2 * SHARD], fp8, kind="ExternalInput"
    )
    WT = nc.dram_tensor("wt", [128, 3, 2, NE * H], fp8, kind="ExternalInput")
    BIA = nc.dram_tensor("bia", [NE, 3 * 2, 128], bf16, kind="ExternalInput")
    IND = nc.dram_tensor("ind", [NE, SHARD], bf16, kind="ExternalInput")
    FC1T = nc.dram_tensor("fc1t", [FCK // FCB, 128, FCB * H], fp8, kind="ExternalInput")
    FC1B = nc.dram_tensor("fc1b", [128, 2], f32, kind="ExternalInput")
    FC2T = nc.dram_tensor("fc2t", [128, 2 * C], bf16, kind="ExternalInput")
    FC2B = nc.dram_tensor("fc2b", [128, 1], f32, kind="ExternalInput")
    OUT = nc.dram_tensor("out", [C, 1], f32, kind="ExternalOutput")

    rg = [list(range(NCORES))]

    with tile.TileContext(nc) as tc:
        with (
            tc.tile_pool(name="wpool", bufs=1) as wpool,
            tc.tile_pool(name="gpool", bufs=2) as gpool,
            tc.tile_pool(name="xpool", bufs=2) as xpool,
            tc.tile_pool(name="atpool", bufs=6) as atpool,
            tc.tile_pool(name="fcpool", bufs=FC_PREFETCH_BUFS) as fcpool,
            tc.tile_pool(name="spool", bufs=2) as spool,
            tc.tile_pool(name="pxp", bufs=3, space="PSUM") as pxp,
            tc.tile_pool(name="pgp", bufs=1, space="PSUM") as pgp,
            tc.tile_pool(name="pzp", bufs=1, space="PSUM") as pzp,
            tc.tile_pool(name="dram", bufs=2, space="DRAM") as dram,
        ):
            # ---- initial G = feat.T + layer weights first (sync queue):
            # these gate the first Wh matmuls; the other small loads go on
            # the gpsimd queue so the first A tiles aren't queued behind.
            g = gpool.tile([128, 2, N_OBJ], fp8, tag="g", name="g_l0")
            nc.sync.dma_start(g[:], G0[:])
            wt_sb = wpool.tile([128, 3, 2, NE * H], fp8)
            # layer-0 weight slice only; layers 1-2 load after the first
            # A tiles so the layer-0 ramp isn't DMA-starved
            nc.sync.dma_start(wt_sb[:, 0], WT[:, 0])

            bia_sb = wpool.tile([NE, 3 * 2, 128], bf16)
            nc.gpsimd.dma_start(bia_sb[:], BIA[:])
            ind_sb = wpool.tile([NE, SHARD], bf16)
            nc.gpsimd.dma_start(ind_sb[:], IND[:])
            fc1b_sb = wpool.tile([128, 2], f32)
            nc.gpsimd.dma_start(fc1b_sb[:], FC1B[:])
            fc2t_sb = wpool.tile([128, 2 * C], bf16)
            nc.gpsimd.dma_start(fc2t_sb[:], FC2T[:])
            fc2b_sb = wpool.tile([128, 1], f32)
            nc.gpsimd.dma_start(fc2b_sb[:], FC2B[:])

            # preload the Sigmoid LUT: a dummy 1-element activation at the
            # start pays the ~1.3us ACT_TABLE_LOAD under layer-0 compute
            # instead of on the critical output path at the end.
            sigwarm = wpool.tile([1, 1], f32)
            nc.scalar.activation(sigwarm[:], fc2b_sb[0:1, 0:1], AF.Sigmoid)

            # fc1 weight tiles: issued spread through layers 1-2 on the
            # scalar engine's DMA queue — its dma_starts sit between px
            # copies in the scalar stream, so issue is throttled by compute
            # progress and never floods the startup or boundary DMA path.
            fc1_tiles = [None] * (FCK // FCB)
            fc1_next = [0]

            def issue_fc1(n=1):
                for _ in range(n):
                    blk = fc1_next[0]
                    if blk >= FCK // FCB:
                        return
                    w16 = fcpool.tile([128, FCB, H], fp8, tag="fc1")
                    nc.scalar.dma_start(
                        w16[:], FC1T[blk].rearrange("p (s f) -> p s f", s=FCB)
                    )
                    fc1_tiles[blk] = w16
                    fc1_next[0] += 1

            # tiny warmup collective: pays the one-time ncfw init + entry
            # barrier (absorbing SPMD launch skew) hidden under layer-0
            # compute, while staying small enough for the Mesh algorithm.
            wuin = dram.tile([1, 256], fp8, tag="wuin")
            nc.gpsimd.dma_start(wuin[:], ATP[0, 0][0:1, 0:256])
            wuout = dram.tile([NCORES, 1, 256], fp8, tag="wuout", addr_space="Shared")
            nc.gpsimd.collective_compute(
                "AllGather",
                mybir.AluOpType.bypass,
                replica_groups=rg,
                ins=[wuin.opt()],
                outs=[wuout.opt()],
            )

            g3sh = None
            for layer in range(n_layers):
                # layer-long PSUM accumulators for out.T = [H, dst_shard]
                pg = [
                    pgp.tile([128, SHARD], f32, tag=f"pg{m}", name=f"pg_l{layer}_{m}")
                    for m in range(2)
                ]
                for pr in range(NE // 2):
                    # Wh for an etype PAIR: one DoubleRow matmul per node
                    # tile contracts all 256 input features at once.
                    x_sb = xpool.tile(
                        [128, NT, 2 * H], fp8, tag="x", name=f"x_l{layer}_p{pr}"
                    )
                    wsl = wt_sb[:, layer, :, 2 * pr * H : 2 * pr * H + 2 * H]
                    for mt in range(NT):
                        px = pxp.tile([128, 512], f32, tag="px")
                        nc.tensor.matmul(
                            px[:],
                            lhsT=g[:, :, mt * 128 : mt * 128 + 128],
                            rhs=wsl,
                            start=True,
                            stop=True,
                            perf_mode=DR,
                        )
                        if mt % 2 == 0:
                            nc.vector.tensor_copy(x_sb[:, mt, :], px[:])
                        else:
                            nc.scalar.copy(x_sb[:, mt, :], px[:])
                    for sub in range(2):
                        e = 2 * pr + sub
                        # A-multiply: pg[m] += X_e[kt-pair].T @ AT_e[kt-pair]
                        for b in range(NP // ABLK):
                            at8 = atpool.tile([128, ABLK, 2, SHARD], fp8, tag="at")
                            nc.sync.dma_start(
                                at8[:],
                                ATP[e, b].rearrange(
                                    "p (s two j) -> p s two j", s=ABLK, two=2
                                ),
                            )
                            for s in range(ABLK):
                                t = b * ABLK + s
                                for m in range(2):
                                    nc.tensor.matmul(
                                        pg[m][:],
                                        lhsT=x_sb[
                                            :,
                                            2 * t : 2 * t + 2,
                                            sub * H + m * 128 : sub * H + m * 128 + 128,
                                        ],
                                        rhs=at8[:, s, :, :],
                                        start=(e == 0 and t == 0),
                                        stop=False,
                                        perf_mode=DR,
                                    )
                        issue_fc1(layer)  # 0/1/2 tiles per etype block
                    if layer == 0 and pr == 0:
                        # balance of the resident weights, after the first
                        # A tiles are in flight
                        nc.sync.dma_start(wt_sb[:, 1:3], WT[:, 1:3])
                # bias: pg[m] += sum_e bia[e, layer, m, :] (x) ind[e, :]
                for m in range(2):
                    nc.tensor.matmul(
                        pg[m][:],
                        lhsT=bia_sb[:, layer * 2 + m, :],
                        rhs=ind_sb[:],
                        start=False,
                        stop=True,
                    )
                if layer < n_layers - 1:
                    # relu -> fp8 shard; PSUM is OUT_SCALE*h', restore H_SCALE.
                    # One AllGather per m-half: 512 KB output stays on the
                    # fast Mesh algorithm (1 MB picks slow RDH), and the
                    # m=0 collective overlaps the m=1 relu/staging.
                    g = gpool.tile([128, 2, N_OBJ], fp8, tag="g", name=f"g_l{layer+1}")
                    for m in range(2):
                        gs = spool.tile(
                            [128, SHARD], fp8, tag=f"gshq{m}", name=f"gshq_l{layer}_{m}"
                        )
                        nc.scalar.activation(
                            gs[:], pg[m][:], AF.Relu, scale=H_SCALE / OUT_SCALE
                        )
                        agin = dram.tile([128, SHARD], fp8, tag=f"agin{m}")
                        nc.gpsimd.dma_start(agin[:], gs[:])
                        agout = dram.tile(
                            [NCORES, 128, SHARD],
                            fp8,
                            tag=f"agout{m}",
                            addr_space="Shared",
                        )
                        nc.gpsimd.collective_compute(
                            "AllGather",
                            mybir.AluOpType.bypass,
                            replica_groups=rg,
                            ins=[agin.opt()],
                            outs=[agout.opt()],
                        )
                        nc.sync.dma_start(
                            g[:, m, :].rearrange("p (c j) -> p c j", c=NCORES),
                            agout[:, :, :].rearrange("c p j -> p c j"),
                        )
                else:
                    g3sh = []
                    for m in range(2):
                        gs = spool.tile(
                            [128, SHARD], bf16, tag=f"gsh{m}", name=f"gsh_l{layer}_{m}"
                        )
                        nc.scalar.activation(
                            gs[:], pg[m][:], AF.Relu, scale=H_SCALE / OUT_SCALE
                        )
                        g3sh.append(gs)

            # ---- fc1: z_partial[1, 256] per PE column group, 4 groups ----
            # M=1 matmuls run 4-wide across 32-col groups of the PE array;
            # partial sums land on PSUM partitions 0/32/64/96.
            pz4 = pzp.tile([128, H], f32, tag="pz4")
            for blk in range(FCK // FCB):
                if fc1_tiles[blk] is None:
                    issue_fc1()
                w16 = fc1_tiles[blk]
                for s in range(FCB):
                    t = blk * FCB + s
                    q = t % 4
                    nc.tensor.matmul(
                        pz4[32 * q : 32 * q + 1, :],
                        lhsT=g3sh[t % 2][:, t // 2 : t // 2 + 1],
                        rhs=w16[:, s, :],
                        start=(t < 4),
                        stop=(t >= FCK - 4),
                        tile_position=(0, 32 * q),
                        skip_group_check=True,
                    )
                issue_fc1()  # keep the DMA stream ahead of consumption
            # copy only the 4 written partial rows (partition-preserving)
            zsb = spool.tile([128, H], f32, tag="zsb")
            for q in range(4):
                if q % 2 == 0:
                    nc.vector.tensor_copy(
                        zsb[32 * q : 32 * q + 1, :], pz4[32 * q : 32 * q + 1, :]
                    )
                else:
                    nc.scalar.copy(
                        zsb[32 * q : 32 * q + 1, :], pz4[32 * q : 32 * q + 1, :]
                    )

            # AllGather the 4 per-core z partials, then sum the 32 rows on
            # the PE with a K=32 ones-matmul — which also transposes z into
            # the [128, 1] column layout fc2 needs.
            agzin = dram.tile([4, H], f32, tag="agzin")
            nc.gpsimd.dma_start(
                agzin[:], zsb[:].rearrange("(a b) h -> a b h", b=32)[:, 0, :]
            )
            agzout = dram.tile([NCORES, 4, H], f32, tag="agzout", addr_space="Shared")
            nc.gpsimd.collective_compute(
                "AllGather",
                mybir.AluOpType.bypass,
                replica_groups=rg,
                ins=[agzin.opt()],
                outs=[agzout.opt()],
            )
            zparts = spool.tile([NCORES * 4, H], f32, tag="zparts")
            nc.sync.dma_start(zparts[:], agzout[:].rearrange("c a h -> (c a) h"))
            ones32 = wpool.tile([NCORES * 4, 1], f32)
            nc.gpsimd.memset(ones32[:], 1.0)

            # NOTE: matmul start=True zeroes the whole 2KB PSUM bank row
            # per touched partition (ZERO_REGION_SIZE), so independent
            # accumulators must live in separate banks.
            po = pzp.tile([C, 1], f32, tag="po")
            pozc = pzp.tile([128, 2], f32, tag="pozc")
            for k in range(2):
                nc.tensor.matmul(
                    pozc[:, k : k + 1],
                    lhsT=zparts[:, k * 128 : (k + 1) * 128],
                    rhs=ones32[:],
                    start=(k == 0),
                    stop=(k == 1),
                    skip_group_check=True,
                )
            for k in range(2):
                zr = spool.tile([128, 1], bf16, tag=f"zr{k}")
                nc.scalar.activation(
                    zr[:],
                    pozc[:, k : k + 1],
                    AF.Relu,
                    bias=fc1b_sb[:, k : k + 1],
                    scale=1.0 / (FC1_SCALE * H_SCALE),
                )
                nc.tensor.matmul(
                    po[:],
                    lhsT=fc2t_sb[:, k * C : (k + 1) * C],
                    rhs=zr[:],
                    start=(k == 0),
                    stop=(k == 1),
                )
            osb = spool.tile([C, 1], f32, tag="osb")
            nc.scalar.activation(osb[:], po[:], AF.Sigmoid, bias=fc2b_sb[:, 0:1])
            nc.gpsimd.dma_start(OUT[:], osb[:])

    nc.compile()
    _split_drain_waits(nc)
    return nc


def _prep_shared(feat, W0, b0, W1, b1, W2, b2, fc1_b, fc2_w, fc2_b):
    """Host layout prep for the tensors every core receives identically."""
    g0 = (
        (np.ascontiguousarray(feat.T) * H_SCALE)
        .astype(FP8)
        .reshape(2, 128, N_OBJ)
        .transpose(1, 0, 2)
    )
    g0 = np.ascontiguousarray(g0)

    # wt[p, layer, k, e*H+h] = W_layer[e][h, k*128+p] * W_SCALE
    wt = np.empty((128, 3, 2, NE * H), dtype=FP8)
    for li, W in enumerate((W0, W1, W2)):
        for e in range(NE):
            wte = np.ascontiguousarray(W[e].T).astype(np.float32) * W_SCALE  # [F, H]
            wt[:, li, 0, e * H : (e + 1) * H] = wte[:128].astype(FP8)
            wt[:, li, 1, e * H : (e + 1) * H] = wte[128:].astype(FP8)

    # bia[e, (layer, m), p] = b_layer[e][m*128+p] * OUT_SCALE
    bia = np.empty((NE, 3 * 2, 128), dtype=BF16)
    for li, b in enumerate((b0, b1, b2)):
        sc = (np.asarray(b, dtype=np.float32) * OUT_SCALE).reshape(NE, 2, 128)
        bia[:, li * 2 : li * 2 + 2, :] = sc.astype(BF16)

    fc1b = np.ascontiguousarray(fc1_b.reshape(2, 128).T).astype(np.float32)
    fc2t = np.ascontiguousarray(
        fc2_w.T.reshape(2, 128, C).transpose(1, 0, 2).reshape(128, 2 * C)
    ).astype(BF16)
    fc2b = fc2_b.reshape(C, 1).astype(np.float32)
    return g0, wt, bia, fc1b, fc2t, fc2b


def _prep_graph(edges):
    """Per-(etype, core) normalized adjacency slices + degree indicators."""
    atp = np.empty((NCORES, NE, NP // ABLK, 128, ABLK * 2 * SHARD), dtype=FP8)
    ind = np.empty((NCORES, NE, SHARD), dtype=BF16)
    for e in range(NE):
        src = np.asarray(edges[e, 0], dtype=np.int64)
        dst = np.asarray(edges[e, 1], dtype=np.int64)
        deg = np.bincount(dst, minlength=N_OBJ).astype(np.float32)
        w = A_SCALE / np.maximum(deg, 1.0)
        an = np.zeros((N_OBJ, N_OBJ), dtype=np.float32)
        np.add.at(an, (src, dst), w[dst])
        an8 = an.astype(FP8)
        ind_e = (deg > 0).astype(BF16)
        for c in range(NCORES):
            sh = an8[:, c * SHARD : (c + 1) * SHARD]  # [4096 src, 512]
            # [NP//ABLK blocks, 128 part, ABLK pairs, 2 planes, 512]
            atp[c, e] = (
                sh.reshape(NP // ABLK, ABLK, 2, 128, SHARD)
                .transpose(0, 3, 1, 2, 4)
                .reshape(NP // ABLK, 128, ABLK * 2 * SHARD)
            )
            ind[c, e] = ind_e[c * SHARD : (c + 1) * SHARD]
    return atp, ind


def _prep_fc1(fc1_w):
    """Per-core column slice of fc1_w, transposed and DMA-batch packed."""
    out = []
    ksl = SHARD * H  # 131072 flat positions per core
    for c in range(NCORES):
        sl = np.ascontiguousarray(fc1_w[:, c * ksl : (c + 1) * ksl].T)  # [131072, 256]
        packed = (
            (sl.reshape(FCK // FCB, FCB, 128, H) * FC1_SCALE)
            .transpose(0, 2, 1, 3)
            .reshape(FCK // FCB, 128, FCB * H)
            .astype(FP8)
        )
        out.append(packed)
    return out


def kernel(feat, edges, W0, b0, W1, b1, W2, b2, fc1_w, fc1_b, fc2_w, fc2_b):
    from concourse.bass_utils import run_bass_kernel_spmd

    if "nc" not in _BASS_CACHE:
        _BASS_CACHE["nc"] = _build_bass()
    nc = _BASS_CACHE["nc"]

    in_maps = _make_in_maps(
        dict(
            feat=feat, edges=edges, W0=W0, b0=b0, W1=W1, b1=b1, W2=W2, b2=b2,
            fc1_w=fc1_w, fc1_b=fc1_b, fc2_w=fc2_w, fc2_b=fc2_b,
        )
    )

    res = run_bass_kernel_spmd(nc, in_maps, core_ids=list(range(NCORES)))
    out = np.asarray(res.results[0]["out"]).reshape(C)
    return out.astype(np.float32)


def _make_in_maps(inputs):
    g0, wt, bia, fc1b, fc2t, fc2b = _prep_shared(
        np.asarray(inputs["feat"], dtype=np.float32),
        np.asarray(inputs["W0"]), np.asarray(inputs["b0"]),
        np.asarray(inputs["W1"]), np.asarray(inputs["b1"]),
        np.asarray(inputs["W2"]), np.asarray(inputs["b2"]),
        np.asarray(inputs["fc1_b"]), np.asarray(inputs["fc2_w"]),
        np.asarray(inputs["fc2_b"]),
    )
    atp, ind = _prep_graph(np.asarray(inputs["edges"]))
    fc1t = _prep_fc1(np.asarray(inputs["fc1_w"]))
    return [
        {
            "g0": g0, "atp": atp[c], "wt": wt, "bia": bia, "ind": ind[c],
            "fc1t": fc1t[c], "fc1b": fc1b, "fc2t": fc2t, "fc2b": fc2b,
        }
        for c in range(NCORES)
    ]


def run_profiled(inputs, trace_cores=None):
    """Test-only: run with NTFF tracing; returns BassKernelResults."""
    from concourse import bass_utils
    from concourse.bass_utils import run_bass_kernel_spmd

    bass_utils.upload_artifacts = lambda tmpdir: f"local://{tmpdir}"
    if "nc" not in _BASS_CACHE:
        _BASS_CACHE["nc"] = _build_bass()
    nc = _BASS_CACHE["nc"]
    in_maps = _make_in_maps(inputs)
    tmpdir = "/tmp/gcn_profile"
    import shutil, os
    shutil.rmtree(tmpdir, ignore_errors=True)
    os.makedirs(tmpdir, exist_ok=True)
    return run_bass_kernel_spmd(
        nc,
        in_maps,
        core_ids=list(range(NCORES)),
        trace=True,
        tmpdir=tmpdir,
        trace_cores=trace_cores,
    )


# revision 52
# speedup vs baseline: 1.0318x; 1.0318x over previous
"""Trainium2 Bass kernel for nn_DGL_GCN (3-layer hetero GCN + MLP head).

Math (reference): 3x hetero layers
    h' = relu( sum_e segment_mean_e( h @ W_e.T + b_e ) )
then z = relu(fc1_w @ h3.flatten() + fc1_b); out = sigmoid(fc2_w @ z + fc2_b).

The per-etype mean aggregation over edges is algebraically
    A_e @ (h @ W_e.T) + ind_e (x) b_e
with A_e[dst, src] = multiplicity(src->dst) / max(deg(dst),1) and
ind_e[dst] = deg(dst) > 0. A_e is a fixed 4096x4096 matrix per etype, so
each layer is dense matmuls on the PE array.

Sharding over 8 cores: destination-node shards (512 dst nodes per core,
all 8 etypes on-core -> all cross-etype sums happen in fp32 PSUM, no
AllReduce needed; one small AllGather of h per layer). fc1 is column-
sharded over the flattened node*hidden dim (each core's own h3 shard is
exactly its fc1 column slice); partial z vectors are AllGather+summed.

All heavy matmuls run fp8 x fp8 with perf_mode=DoubleRow (K=256 per
pass, 2 MACs/cell/cycle) accumulating in fp32 PSUM. fc1's thin-M
(M=1) matmuls are packed 4-wide across PE column groups via
tile_position so four run concurrently.
"""

import numpy as np
import ml_dtypes

N_OBJ = 4096
F_IN = 256
H = 256
C = 128
NE = 8
NCORES = 8
SHARD = N_OBJ // NCORES          # 512 dst nodes per core
NT = N_OBJ // 128                # 32 node tiles
NP = NT // 2                     # 16 k-tile pairs (DoubleRow)
ABLK = 4                         # kt-pairs batched per A DMA
FCB = 16                         # fc1 k-tiles batched per DMA
FCK = (SHARD * H) // 128         # 1024 fc1 k-tiles per core
FC_PREFETCH_BUFS = 29            # fc1 tiles resident during layers

BF16 = ml_dtypes.bfloat16
FP8 = ml_dtypes.float8_e4m3
FC1_SCALE = 8192.0  # fc1_w ~N(0, 0.002) is subnormal in e4m3; pre-scale
H_SCALE = 16.0      # hidden state h kept as 16*h in fp8
W_SCALE = 4.0       # per-etype W kept as 4*W.T in fp8 (keeps X < 240)
A_SCALE = 4.0       # adjacency kept as 4*A in fp8
OUT_SCALE = H_SCALE * W_SCALE * A_SCALE  # PSUM domain of a layer output

_BASS_CACHE = {}


def _split_drain_waits(nc, max_waits=1):
    # This walrus build accepts only one sync-wait command on an InstDrain;
    # Tile's tail drain waits on every active proc lane. Split into a chain
    # of single-wait drains.
    import copy
    import concourse.mybir as mybir

    for f in nc.m.functions:
        for bb in f.blocks:
            new_list = []
            for ins in bb.instructions:
                si = ins.sync_info
                if (
                    isinstance(ins, mybir.InstDrain)
                    and si is not None
                    and si.on_wait
                    and len(si.on_wait) > max_waits
                ):
                    waits = list(si.on_wait)
                    updates = list(si.on_update or [])
                    for i, w in enumerate(waits[:-1]):
                        d = copy.deepcopy(ins)
                        d.name = f"{ins.name}-sw{i}"
                        dsi = d.sync_info
                        dsi.on_wait = [w]
                        dsi.on_update = []
                        d.sync_info = dsi
                        new_list.append(d)
                        nc.inst_map[d.name] = d
                    si.on_wait = [waits[-1]]
                    si.on_update = updates
                    ins.sync_info = si
                new_list.append(ins)
            bb.instructions[:] = new_list


def _build_bass(n_layers=3):
    import concourse.bass as bass  # noqa: F401
    import concourse.tile as tile
    import concourse.mybir as mybir
    from concourse import bacc

    f32 = mybir.dt.float32
    bf16 = mybir.dt.bfloat16
    fp8 = mybir.dt.float8e4
    AF = mybir.ActivationFunctionType
    DR = mybir.MatmulPerfMode.DoubleRow

    nc = bacc.Bacc(
        "TRN2", target_bir_lowering=False, debug=False, num_devices=NCORES
    )

    # ---- I/O (per-core values supplied via in_maps) ----
    G0 = nc.dram_tensor("g0", [128, 2, N_OBJ], fp8, kind="ExternalInput")
    ATP = nc.dram_tensor(
        "atp", [NE, NP // ABLK, 128, ABLK * 2 * SHARD], fp8, kind="ExternalInput"
    )
    WT = nc.dram_tensor("wt", [128, 3, 2, NE * H], fp8, kind="ExternalInput")
    BIA = nc.dram_tensor("bia", [NE, 3 * 2, 128], bf16, kind="ExternalInput")
    IND = nc.dram_tensor("ind", [NE, SHARD], bf16, kind="ExternalInput")
    FC1T = nc.dram_tensor("fc1t", [FCK // FCB, 128, FCB * H], fp8, kind="ExternalInput")
    FC1B = nc.dram_tensor("fc1b", [128, 2], f32, kind="ExternalInput")
    FC2T = nc.dram_tensor("fc2t", [128, 2 * C], bf16, kind="ExternalInput")
    FC2B = nc.dram_tensor("fc2b", [128, 1], f32, kind="ExternalInput")
    OUT = nc.dram_tensor("out", [C, 1], f32, kind="ExternalOutput")

    rg = [list(range(NCORES))]

    with tile.TileContext(nc) as tc:
        with (
            tc.tile_pool(name="wpool", bufs=1) as wpool,
            tc.tile_pool(name="gpool", bufs=1) as gpool,
            tc.tile_pool(name="xpool", bufs=2) as xpool,
            tc.tile_pool(name="atpool", bufs=5) as atpool,
            tc.tile_pool(name="fcpool", bufs=FC_PREFETCH_BUFS) as fcpool,
            tc.tile_pool(name="spool", bufs=2) as spool,
            tc.tile_pool(name="pxp", bufs=4, space="PSUM") as pxp,
            tc.tile_pool(name="pgp", bufs=1, space="PSUM") as pgp,
            tc.tile_pool(name="pzp", bufs=1, space="PSUM") as pzp,
            tc.tile_pool(name="dram", bufs=2, space="DRAM") as dram,
        ):
            # ---- initial G = feat.T + layer weights first (sync queue):
            # these gate the first Wh matmuls; the other small loads go on
            # the gpsimd queue so the first A tiles aren't queued behind.
            g = gpool.tile([128, 2, N_OBJ], fp8, tag="g", name="g_l0")
            nc.sync.dma_start(g[:], G0[:])
            wt_sb = wpool.tile([128, 3, 2, NE * H], fp8)
            # layer-0 weight slice only; layers 1-2 load after the first
            # A tiles so the layer-0 ramp isn't DMA-starved
            nc.sync.dma_start(wt_sb[:, 0], WT[:, 0])

            bia_sb = wpool.tile([NE, 3 * 2, 128], bf16)
            nc.gpsimd.dma_start(bia_sb[:], BIA[:])
            ind_sb = wpool.tile([NE, SHARD], bf16)
            nc.gpsimd.dma_start(ind_sb[:], IND[:])
            fc1b_sb = wpool.tile([128, 2], f32)
            nc.gpsimd.dma_start(fc1b_sb[:], FC1B[:])
            fc2t_sb = wpool.tile([128, 2 * C], bf16)
            nc.gpsimd.dma_start(fc2t_sb[:], FC2T[:])
            fc2b_sb = wpool.tile([128, 1], f32)
            nc.gpsimd.dma_start(fc2b_sb[:], FC2B[:])

            # preload the Sigmoid LUT: a dummy 1-element activation at the
            # start pays the ~1.3us ACT_TABLE_LOAD under layer-0 compute
            # instead of on the critical output path at the end.
            sigwarm = wpool.tile([1, 1], f32)
            nc.scalar.activation(sigwarm[:], fc2b_sb[0:1, 0:1], AF.Sigmoid)

            # fc1 weight tiles: issued spread through layers 1-2 on the
            # scalar engine's DMA queue — its dma_starts sit between px
            # copies in the scalar stream, so issue is throttled by compute
            # progress and never floods the startup or boundary DMA path.
            fc1_tiles = [None] * (FCK // FCB)
            fc1_next = [0]

            def issue_fc1(n=1):
                for _ in range(n):
                    blk = fc1_next[0]
                    if blk >= FCK // FCB:
                        return
                    w16 = fcpool.tile([128, FCB, H], fp8, tag="fc1")
                    nc.scalar.dma_start(
                        w16[:], FC1T[blk].rearrange("p (s f) -> p s f", s=FCB)
                    )
                    fc1_tiles[blk] = w16
                    fc1_next[0] += 1

            # tiny warmup collective: pays the one-time ncfw init + entry
            # barrier (absorbing SPMD launch skew) hidden under layer-0
            # compute, while staying small enough for the Mesh algorithm.
            wuin = dram.tile([1, 256], fp8, tag="wuin")
            nc.gpsimd.dma_start(wuin[:], ATP[0, 0][0:1, 0:256])
            wuout = dram.tile([NCORES, 1, 256], fp8, tag="wuout", addr_space="Shared")
            nc.gpsimd.collective_compute(
                "AllGather",
                mybir.AluOpType.bypass,
                replica_groups=rg,
                ins=[wuin.opt()],
                outs=[wuout.opt()],
            )

            g3sh = None
            for layer in range(n_layers):
                # layer-long PSUM accumulators for out.T = [H, dst_shard]
                pg = [
                    pgp.tile([128, SHARD], f32, tag=f"pg{m}", name=f"pg_l{layer}_{m}")
                    for m in range(2)
                ]
                for pr in range(NE // 2):
                    # Wh for an etype PAIR: one DoubleRow matmul per node
                    # tile contracts all 256 input features at once.
                    x_sb = xpool.tile(
                        [128, NT, 2 * H], fp8, tag="x", name=f"x_l{layer}_p{pr}"
                    )
                    wsl = wt_sb[:, layer, :, 2 * pr * H : 2 * pr * H + 2 * H]
                    for mt in range(NT):
                        px = pxp.tile([128, 512], f32, tag="px")
                        nc.tensor.matmul(
                            px[:],
                            lhsT=g[:, :, mt * 128 : mt * 128 + 128],
                            rhs=wsl,
                            start=True,
                            stop=True,
                            perf_mode=DR,
                        )
                        if mt % 2 == 0:
                            nc.vector.tensor_copy(x_sb[:, mt, :], px[:])
                        else:
                            nc.scalar.copy(x_sb[:, mt, :], px[:])
                    for sub in range(2):
                        e = 2 * pr + sub
                        # A-multiply: pg[m] += X_e[kt-pair].T @ AT_e[kt-pair]
                        for b in range(NP // ABLK):
                            at8 = atpool.tile([128, ABLK, 2, SHARD], fp8, tag="at")
                            nc.sync.dma_start(
                                at8[:],
                                ATP[e, b].rearrange(
                                    "p (s two j) -> p s two j", s=ABLK, two=2
                                ),
                            )
                            for s in range(ABLK):
                                t = b * ABLK + s
                                for m in range(2):
                                    nc.tensor.matmul(
                                        pg[m][:],
                                        lhsT=x_sb[
                                            :,
                                            2 * t : 2 * t + 2,
                                            sub * H + m * 128 : sub * H + m * 128 + 128,
                                        ],
                                        rhs=at8[:, s, :, :],
                                        start=(e == 0 and t == 0),
                                        stop=False,
                                        perf_mode=DR,
                                    )
                        issue_fc1(layer)  # 0/1/2 tiles per etype block
                    if layer == 0 and pr == 0:
                        # balance of the resident weights, after the first
                        # A tiles are in flight
                        nc.sync.dma_start(wt_sb[:, 1:3], WT[:, 1:3])
                # bias: pg[m] += sum_e bia[e, layer, m, :] (x) ind[e, :]
                for m in range(2):
                    nc.tensor.matmul(
                        pg[m][:],
                        lhsT=bia_sb[:, layer * 2 + m, :],
                        rhs=ind_sb[:],
                        start=False,
                        stop=True,
                    )
                if layer < n_layers - 1:
                    # relu -> fp8 shard; PSUM is OUT_SCALE*h', restore H_SCALE.
                    # One AllGather per m-half: 512 KB output stays on the
                    # fast Mesh algorithm (1 MB picks slow RDH), and the
                    # m=0 collective overlaps the m=1 relu/staging.
                    g = gpool.tile([128, 2, N_OBJ], fp8, tag="g", name=f"g_l{layer+1}")
                    for m in range(2):
                        gs = spool.tile(
                            [128, SHARD], fp8, tag=f"gshq{m}", name=f"gshq_l{layer}_{m}"
                        )
                        nc.scalar.activation(
                            gs[:], pg[m][:], AF.Relu, scale=H_SCALE / OUT_SCALE
                        )
                        agin = dram.tile([128, SHARD], fp8, tag=f"agin{m}")
                        nc.gpsimd.dma_start(agin[:], gs[:])
                        agout = dram.tile(
                            [NCORES, 128, SHARD],
                            fp8,
                            tag=f"agout{m}",
                            addr_space="Shared",
                        )
                        nc.gpsimd.collective_compute(
                            "AllGather",
                            mybir.AluOpType.bypass,
                            replica_groups=rg,
                            ins=[agin.opt()],
                            outs=[agout.opt()],
                        )
                        nc.sync.dma_start(
                            g[:, m, :].rearrange("p (c j) -> p c j", c=NCORES),
                            agout[:, :, :].rearrange("c p j -> p c j"),
                        )
                else:
                    g3sh = []
                    for m in range(2):
                        gs = spool.tile(
                            [128, SHARD], bf16, tag=f"gsh{m}", name=f"gsh_l{layer}_{m}"
                        )
                        nc.scalar.activation(
                            gs[:], pg[m][:], AF.Relu, scale=H_SCALE / OUT_SCALE
                        )
                        g3sh.append(gs)

            # ---- fc1: z_partial[1, 256] per PE column group, 4 groups ----
            # M=1 matmuls run 4-wide across 32-col groups of the PE array;
            # partial sums land on PSUM partitions 0/32/64/96.
            # reuse a (dead) px PSUM buffer for the z partials: frees a
            # pzp bank so pxp can run 4 buffers during the layers. The 4
            # col-group chains touch disjoint partitions, so start=True's
            # bank-row zeroing stays safe.
            pz4 = pxp.tile([128, 512], f32, tag="px")
            for blk in range(FCK // FCB):
                if fc1_tiles[blk] is None:
                    issue_fc1()
                w16 = fc1_tiles[blk]
                for s in range(FCB):
                    t = blk * FCB + s
                    q = t % 4
                    nc.tensor.matmul(
                        pz4[32 * q : 32 * q + 1, 0:H],
                        lhsT=g3sh[t % 2][:, t // 2 : t // 2 + 1],
                        rhs=w16[:, s, :],
                        start=(t < 4),
                        stop=(t >= FCK - 4),
                        tile_position=(0, 32 * q),
                        skip_group_check=True,
                    )
                issue_fc1()  # keep the DMA stream ahead of consumption
            # copy only the 4 written partial rows (partition-preserving)
            zsb = spool.tile([128, H], f32, tag="zsb")
            for q in range(4):
                if q % 2 == 0:
                    nc.vector.tensor_copy(
                        zsb[32 * q : 32 * q + 1, :], pz4[32 * q : 32 * q + 1, 0:H]
                    )
                else:
                    nc.scalar.copy(
                        zsb[32 * q : 32 * q + 1, :], pz4[32 * q : 32 * q + 1, 0:H]
                    )

            # AllGather the 4 per-core z partials, then sum the 32 rows on
            # the PE with a K=32 ones-matmul — which also transposes z into
            # the [128, 1] column layout fc2 needs.
            agzin = dram.tile([4, H], f32, tag="agzin")
            nc.gpsimd.dma_start(
                agzin[:], zsb[:].rearrange("(a b) h -> a b h", b=32)[:, 0, :]
            )
            agzout = dram.tile([NCORES, 4, H], f32, tag="agzout", addr_space="Shared")
            nc.gpsimd.collective_compute(
                "AllGather",
                mybir.AluOpType.bypass,
                replica_groups=rg,
                ins=[agzin.opt()],
                outs=[agzout.opt()],
            )
            zparts = spool.tile([NCORES * 4, H], f32, tag="zparts")
            nc.sync.dma_start(zparts[:], agzout[:].rearrange("c a h -> (c a) h"))
            ones32 = wpool.tile([NCORES * 4, 1], f32)
            nc.gpsimd.memset(ones32[:], 1.0)

            # NOTE: matmul start=True zeroes the whole 2KB PSUM bank row
            # per touched partition (ZERO_REGION_SIZE), so independent
            # accumulators must live in separate banks.
            po = pzp.tile([C, 1], f32, tag="po")
            pozc = pzp.tile([128, 2], f32, tag="pozc")
            for k in range(2):
                nc.tensor.matmul(
                    pozc[:, k : k + 1],
                    lhsT=zparts[:, k * 128 : (k + 1) * 128],
                    rhs=ones32[:],
                    start=(k == 0),
                    stop=(k == 1),
                    skip_group_check=True,
                )
            for k in range(2):
                zr = spool.tile([128, 1], bf16, tag=f"zr{k}")
                nc.scalar.activation(
                    zr[:],
                    pozc[:, k : k + 1],
                    AF.Relu,
                    bias=fc1b_sb[:, k : k + 1],
                    scale=1.0 / (FC1_SCALE * H_SCALE),
                )
                nc.tensor.matmul(
                    po[:],
                    lhsT=fc2t_sb[:, k * C : (k + 1) * C],
                    rhs=zr[:],
                    start=(k == 0),
                    stop=(k == 1),
                )
            osb = spool.tile([C, 1], f32, tag="osb")
            nc.scalar.activation(osb[:], po[:], AF.Sigmoid, bias=fc2b_sb[:, 0:1])
            # sync queue: the gpsimd queue's tail drain is ~7us slower
            nc.sync.dma_start(OUT[:], osb[:])

    nc.compile()
    _split_drain_waits(nc)
    return nc


def _prep_shared(feat, W0, b0, W1, b1, W2, b2, fc1_b, fc2_w, fc2_b):
    """Host layout prep for the tensors every core receives identically."""
    g0 = (
        (np.ascontiguousarray(feat.T) * H_SCALE)
        .astype(FP8)
        .reshape(2, 128, N_OBJ)
        .transpose(1, 0, 2)
    )
    g0 = np.ascontiguousarray(g0)

    # wt[p, layer, k, e*H+h] = W_layer[e][h, k*128+p] * W_SCALE
    wt = np.empty((128, 3, 2, NE * H), dtype=FP8)
    for li, W in enumerate((W0, W1, W2)):
        for e in range(NE):
            wte = np.ascontiguousarray(W[e].T).astype(np.float32) * W_SCALE  # [F, H]
            wt[:, li, 0, e * H : (e + 1) * H] = wte[:128].astype(FP8)
            wt[:, li, 1, e * H : (e + 1) * H] = wte[128:].astype(FP8)

    # bia[e, (layer, m), p] = b_layer[e][m*128+p] * OUT_SCALE
    bia = np.empty((NE, 3 * 2, 128), dtype=BF16)
    for li, b in enumerate((b0, b1, b2)):
        sc = (np.asarray(b, dtype=np.float32) * OUT_SCALE).reshape(NE, 2, 128)
        bia[:, li * 2 : li * 2 + 2, :] = sc.astype(BF16)

    fc1b = np.ascontiguousarray(fc1_b.reshape(2, 128).T).astype(np.float32)
    fc2t = np.ascontiguousarray(
        fc2_w.T.reshape(2, 128, C).transpose(1, 0, 2).reshape(128, 2 * C)
    ).astype(BF16)
    fc2b = fc2_b.reshape(C, 1).astype(np.float32)
    return g0, wt, bia, fc1b, fc2t, fc2b


def _prep_graph(edges):
    """Per-(etype, core) normalized adjacency slices + degree indicators."""
    atp = np.empty((NCORES, NE, NP // ABLK, 128, ABLK * 2 * SHARD), dtype=FP8)
    ind = np.empty((NCORES, NE, SHARD), dtype=BF16)
    for e in range(NE):
        src = np.asarray(edges[e, 0], dtype=np.int64)
        dst = np.asarray(edges[e, 1], dtype=np.int64)
        deg = np.bincount(dst, minlength=N_OBJ).astype(np.float32)
        w = A_SCALE / np.maximum(deg, 1.0)
        an = np.zeros((N_OBJ, N_OBJ), dtype=np.float32)
        np.add.at(an, (src, dst), w[dst])
        an8 = an.astype(FP8)
        ind_e = (deg > 0).astype(BF16)
        for c in range(NCORES):
            sh = an8[:, c * SHARD : (c + 1) * SHARD]  # [4096 src, 512]
            # [NP//ABLK blocks, 128 part, ABLK pairs, 2 planes, 512]
            atp[c, e] = (
                sh.reshape(NP // ABLK, ABLK, 2, 128, SHARD)
                .transpose(0, 3, 1, 2, 4)
                .reshape(NP // ABLK, 128, ABLK * 2 * SHARD)
            )
            ind[c, e] = ind_e[c * SHARD : (c + 1) * SHARD]
    return atp, ind


def _prep_fc1(fc1_w):
    """Per-core column slice of fc1_w, transposed and DMA-batch packed."""
    out = []
    ksl = SHARD * H  # 131072 flat positions per core
    for c in range(NCORES):
        sl = np.ascontiguousarray(fc1_w[:, c * ksl : (c + 1) * ksl].T)  # [131072, 256]
        packed = (
            (sl.reshape(FCK // FCB, FCB, 128, H) * FC1_SCALE)
            .transpose(0, 2, 1, 3)
            .reshape(FCK // FCB, 128, FCB * H)
            .astype(FP8)
        )
        out.append(packed)
    return out


def kernel(feat, edges, W0, b0, W1, b1, W2, b2, fc1_w, fc1_b, fc2_w, fc2_b):
    from concourse.bass_utils import run_bass_kernel_spmd

    if "nc" not in _BASS_CACHE:
        _BASS_CACHE["nc"] = _build_bass()
    nc = _BASS_CACHE["nc"]

    in_maps = _make_in_maps(
        dict(
            feat=feat, edges=edges, W0=W0, b0=b0, W1=W1, b1=b1, W2=W2, b2=b2,
            fc1_w=fc1_w, fc1_b=fc1_b, fc2_w=fc2_w, fc2_b=fc2_b,
        )
    )

    res = run_bass_kernel_spmd(nc, in_maps, core_ids=list(range(NCORES)))
    out = np.asarray(res.results[0]["out"]).reshape(C)
    return out.astype(np.float32)


def _make_in_maps(inputs):
    g0, wt, bia, fc1b, fc2t, fc2b = _prep_shared(
        np.asarray(inputs["feat"], dtype=np.float32),
        np.asarray(inputs["W0"]), np.asarray(inputs["b0"]),
        np.asarray(inputs["W1"]), np.asarray(inputs["b1"]),
        np.asarray(inputs["W2"]), np.asarray(inputs["b2"]),
        np.asarray(inputs["fc1_b"]), np.asarray(inputs["fc2_w"]),
        np.asarray(inputs["fc2_b"]),
    )
    atp, ind = _prep_graph(np.asarray(inputs["edges"]))
    fc1t = _prep_fc1(np.asarray(inputs["fc1_w"]))
    return [
        {
            "g0": g0, "atp": atp[c], "wt": wt, "bia": bia, "ind": ind[c],
            "fc1t": fc1t[c], "fc1b": fc1b, "fc2t": fc2t, "fc2b": fc2b,
        }
        for c in range(NCORES)
    ]


def run_profiled(inputs, trace_cores=None):
    """Test-only: run with NTFF tracing; returns BassKernelResults."""
    from concourse import bass_utils
    from concourse.bass_utils import run_bass_kernel_spmd

    bass_utils.upload_artifacts = lambda tmpdir: f"local://{tmpdir}"
    if "nc" not in _BASS_CACHE:
        _BASS_CACHE["nc"] = _build_bass()
    nc = _BASS_CACHE["nc"]
    in_maps = _make_in_maps(inputs)
    tmpdir = "/tmp/gcn_profile"
    import shutil, os
    shutil.rmtree(tmpdir, ignore_errors=True)
    os.makedirs(tmpdir, exist_ok=True)
    return run_bass_kernel_spmd(
        nc,
        in_maps,
        core_ids=list(range(NCORES)),
        trace=True,
        tmpdir=tmpdir,
        trace_cores=trace_cores,
    )


# revision 55
# speedup vs baseline: 1.1194x; 1.0848x over previous
"""Trainium2 Bass kernel for nn_DGL_GCN (3-layer hetero GCN + MLP head).

Math (reference): 3x hetero layers
    h' = relu( sum_e segment_mean_e( h @ W_e.T + b_e ) )
then z = relu(fc1_w @ h3.flatten() + fc1_b); out = sigmoid(fc2_w @ z + fc2_b).

The per-etype mean aggregation over edges is algebraically
    A_e @ (h @ W_e.T) + ind_e (x) b_e
with A_e[dst, src] = multiplicity(src->dst) / max(deg(dst),1) and
ind_e[dst] = deg(dst) > 0. A_e is a fixed 4096x4096 matrix per etype, so
each layer is dense matmuls on the PE array.

Sharding over 8 cores: destination-node shards (512 dst nodes per core,
all 8 etypes on-core -> all cross-etype sums happen in fp32 PSUM, no
AllReduce needed; one small AllGather of h per layer). fc1 is column-
sharded over the flattened node*hidden dim (each core's own h3 shard is
exactly its fc1 column slice); partial z vectors are AllGather+summed.

All heavy matmuls run fp8 x fp8 with perf_mode=DoubleRow (K=256 per
pass, 2 MACs/cell/cycle) accumulating in fp32 PSUM. fc1's thin-M
(M=1) matmuls are packed 4-wide across PE column groups via
tile_position so four run concurrently.
"""

import numpy as np
import ml_dtypes

N_OBJ = 4096
F_IN = 256
H = 256
C = 128
NE = 8
NCORES = 8
SHARD = N_OBJ // NCORES          # 512 dst nodes per core
NT = N_OBJ // 128                # 32 node tiles
NP = NT // 2                     # 16 k-tile pairs (DoubleRow)
ABLK = 8                         # kt-pairs batched per A DMA
FCB = 16                         # fc1 k-tiles batched per DMA
FCK = (SHARD * H) // 128         # 1024 fc1 k-tiles per core
FC_PREFETCH_BUFS = 28            # fc1 tiles resident during layers

BF16 = ml_dtypes.bfloat16
FP8 = ml_dtypes.float8_e4m3
FC1_SCALE = 8192.0  # fc1_w ~N(0, 0.002) is subnormal in e4m3; pre-scale
H_SCALE = 16.0      # hidden state h kept as 16*h in fp8
W_SCALE = 4.0       # per-etype W kept as 4*W.T in fp8 (keeps X < 240)
A_SCALE = 4.0       # adjacency kept as 4*A in fp8
OUT_SCALE = H_SCALE * W_SCALE * A_SCALE  # PSUM domain of a layer output

_BASS_CACHE = {}


def _split_drain_waits(nc, max_waits=1):
    # This walrus build accepts only one sync-wait command on an InstDrain;
    # Tile's tail drain waits on every active proc lane. Split into a chain
    # of single-wait drains.
    import copy
    import concourse.mybir as mybir

    for f in nc.m.functions:
        for bb in f.blocks:
            new_list = []
            for ins in bb.instructions:
                si = ins.sync_info
                if (
                    isinstance(ins, mybir.InstDrain)
                    and si is not None
                    and si.on_wait
                    and len(si.on_wait) > max_waits
                ):
                    waits = list(si.on_wait)
                    updates = list(si.on_update or [])
                    for i, w in enumerate(waits[:-1]):
                        d = copy.deepcopy(ins)
                        d.name = f"{ins.name}-sw{i}"
                        dsi = d.sync_info
                        dsi.on_wait = [w]
                        dsi.on_update = []
                        d.sync_info = dsi
                        new_list.append(d)
                        nc.inst_map[d.name] = d
                    si.on_wait = [waits[-1]]
                    si.on_update = updates
                    ins.sync_info = si
                new_list.append(ins)
            bb.instructions[:] = new_list


def _build_bass(n_layers=3):
    import concourse.bass as bass  # noqa: F401
    import concourse.tile as tile
    import concourse.mybir as mybir
    from concourse import bacc

    f32 = mybir.dt.float32
    bf16 = mybir.dt.bfloat16
    fp8 = mybir.dt.float8e4
    AF = mybir.ActivationFunctionType
    DR = mybir.MatmulPerfMode.DoubleRow

    nc = bacc.Bacc(
        "TRN2", target_bir_lowering=False, debug=False, num_devices=NCORES
    )

    # ---- I/O (per-core values supplied via in_maps) ----
    G0 = nc.dram_tensor("g0", [128, 2, N_OBJ], fp8, kind="ExternalInput")
    ATP = nc.dram_tensor(
        "atp", [NE, NP // ABLK, 128, ABLK * 2 * SHARD], fp8, kind="ExternalInput"
    )
    WT = nc.dram_tensor("wt", [128, 3, 2, NE * H], fp8, kind="ExternalInput")
    BIA = nc.dram_tensor("bia", [NE, 3 * 2, 128], bf16, kind="ExternalInput")
    IND = nc.dram_tensor("ind", [NE, SHARD], bf16, kind="ExternalInput")
    FC1T = nc.dram_tensor("fc1t", [FCK // FCB, 128, FCB * H], fp8, kind="ExternalInput")
    FC1B = nc.dram_tensor("fc1b", [128, 2], f32, kind="ExternalInput")
    FC2T = nc.dram_tensor("fc2t", [128, 2 * C], bf16, kind="ExternalInput")
    FC2B = nc.dram_tensor("fc2b", [128, 1], f32, kind="ExternalInput")
    OUT = nc.dram_tensor("out", [C, 1], f32, kind="ExternalOutput")

    rg = [list(range(NCORES))]

    with tile.TileContext(nc) as tc:
        with (
            tc.tile_pool(name="wpool", bufs=1) as wpool,
            tc.tile_pool(name="gpool", bufs=1) as gpool,
            tc.tile_pool(name="xpool", bufs=2) as xpool,
            tc.tile_pool(name="atpool", bufs=3) as atpool,
            tc.tile_pool(name="fcpool", bufs=FC_PREFETCH_BUFS) as fcpool,
            tc.tile_pool(name="spool", bufs=2) as spool,
            tc.tile_pool(name="pxp", bufs=4, space="PSUM") as pxp,
            tc.tile_pool(name="pgp", bufs=1, space="PSUM") as pgp,
            tc.tile_pool(name="pzp", bufs=1, space="PSUM") as pzp,
            tc.tile_pool(name="dram", bufs=2, space="DRAM") as dram,
        ):
            # ---- initial G = feat.T + layer weights first (sync queue):
            # these gate the first Wh matmuls; the other small loads go on
            # the gpsimd queue so the first A tiles aren't queued behind.
            g = gpool.tile([128, 2, N_OBJ], fp8, tag="g", name="g_l0")
            nc.sync.dma_start(g[:], G0[:])
            wt_sb = wpool.tile([128, 3, 2, NE * H], fp8)
            # layer-0 weight slice only; layers 1-2 load after the first
            # A tiles so the layer-0 ramp isn't DMA-starved
            nc.sync.dma_start(wt_sb[:, 0], WT[:, 0])

            bia_sb = wpool.tile([NE, 3 * 2, 128], bf16)
            nc.gpsimd.dma_start(bia_sb[:], BIA[:])
            ind_sb = wpool.tile([NE, SHARD], bf16)
            nc.gpsimd.dma_start(ind_sb[:], IND[:])
            fc1b_sb = wpool.tile([128, 2], f32)
            nc.gpsimd.dma_start(fc1b_sb[:], FC1B[:])
            fc2t_sb = wpool.tile([128, 2 * C], bf16)
            nc.gpsimd.dma_start(fc2t_sb[:], FC2T[:])
            fc2b_sb = wpool.tile([128, 1], f32)
            nc.gpsimd.dma_start(fc2b_sb[:], FC2B[:])

            # preload the Sigmoid LUT: a dummy 1-element activation at the
            # start pays the ~1.3us ACT_TABLE_LOAD under layer-0 compute
            # instead of on the critical output path at the end.
            sigwarm = wpool.tile([1, 1], f32)
            nc.scalar.activation(sigwarm[:], fc2b_sb[0:1, 0:1], AF.Sigmoid)

            # fc1 weight tiles: issued spread through layers 1-2 on the
            # scalar engine's DMA queue — its dma_starts sit between px
            # copies in the scalar stream, so issue is throttled by compute
            # progress and never floods the startup or boundary DMA path.
            fc1_tiles = [None] * (FCK // FCB)
            fc1_next = [0]

            def issue_fc1(n=1):
                for _ in range(n):
                    blk = fc1_next[0]
                    if blk >= FCK // FCB:
                        return
                    w16 = fcpool.tile([128, FCB, H], fp8, tag="fc1")
                    nc.scalar.dma_start(
                        w16[:], FC1T[blk].rearrange("p (s f) -> p s f", s=FCB)
                    )
                    fc1_tiles[blk] = w16
                    fc1_next[0] += 1

            # tiny warmup collective: pays the one-time ncfw init + entry
            # barrier (absorbing SPMD launch skew) hidden under layer-0
            # compute, while staying small enough for the Mesh algorithm.
            wuin = dram.tile([1, 256], fp8, tag="wuin")
            nc.gpsimd.dma_start(wuin[:], ATP[0, 0][0:1, 0:256])
            wuout = dram.tile([NCORES, 1, 256], fp8, tag="wuout", addr_space="Shared")
            nc.gpsimd.collective_compute(
                "AllGather",
                mybir.AluOpType.bypass,
                replica_groups=rg,
                ins=[wuin.opt()],
                outs=[wuout.opt()],
            )

            g3sh = None
            for layer in range(n_layers):
                # layer-long PSUM accumulators for out.T = [H, dst_shard]
                pg = [
                    pgp.tile([128, SHARD], f32, tag=f"pg{m}", name=f"pg_l{layer}_{m}")
                    for m in range(2)
                ]
                for pr in range(NE // 2):
                    # Wh for an etype PAIR: one DoubleRow matmul per node
                    # tile contracts all 256 input features at once.
                    x_sb = xpool.tile(
                        [128, NT, 2 * H], fp8, tag="x", name=f"x_l{layer}_p{pr}"
                    )
                    wsl = wt_sb[:, layer, :, 2 * pr * H : 2 * pr * H + 2 * H]
                    for mt in range(NT):
                        px = pxp.tile([128, 512], f32, tag="px")
                        nc.tensor.matmul(
                            px[:],
                            lhsT=g[:, :, mt * 128 : mt * 128 + 128],
                            rhs=wsl,
                            start=True,
                            stop=True,
                            perf_mode=DR,
                        )
                        if mt % 2 == 0:
                            nc.vector.tensor_copy(x_sb[:, mt, :], px[:])
                        else:
                            nc.scalar.copy(x_sb[:, mt, :], px[:])
                    for sub in range(2):
                        e = 2 * pr + sub
                        # A-multiply: pg[m] += X_e[kt-pair].T @ AT_e[kt-pair]
                        for b in range(NP // ABLK):
                            at8 = atpool.tile([128, ABLK, 2, SHARD], fp8, tag="at")
                            nc.sync.dma_start(
                                at8[:],
                                ATP[e, b].rearrange(
                                    "p (s two j) -> p s two j", s=ABLK, two=2
                                ),
                            )
                            for s in range(ABLK):
                                t = b * ABLK + s
                                for m in range(2):
                                    nc.tensor.matmul(
                                        pg[m][:],
                                        lhsT=x_sb[
                                            :,
                                            2 * t : 2 * t + 2,
                                            sub * H + m * 128 : sub * H + m * 128 + 128,
                                        ],
                                        rhs=at8[:, s, :, :],
                                        start=(e == 0 and t == 0),
                                        stop=False,
                                        perf_mode=DR,
                                    )
                        issue_fc1(layer)  # 0/1/2 tiles per etype block
                    if layer == 0 and pr == 0:
                        # balance of the resident weights, after the first
                        # A tiles are in flight
                        nc.sync.dma_start(wt_sb[:, 1:3], WT[:, 1:3])
                # bias: pg[m] += sum_e bia[e, layer, m, :] (x) ind[e, :]
                for m in range(2):
                    nc.tensor.matmul(
                        pg[m][:],
                        lhsT=bia_sb[:, layer * 2 + m, :],
                        rhs=ind_sb[:],
                        start=False,
                        stop=True,
                    )
                if layer < n_layers - 1:
                    # relu -> fp8 shard; PSUM is OUT_SCALE*h', restore H_SCALE.
                    # One AllGather per m-half: 512 KB output stays on the
                    # fast Mesh algorithm (1 MB picks slow RDH), and the
                    # m=0 collective overlaps the m=1 relu/staging.
                    g = gpool.tile([128, 2, N_OBJ], fp8, tag="g", name=f"g_l{layer+1}")
                    for m in range(2):
                        gs = spool.tile(
                            [128, SHARD], fp8, tag=f"gshq{m}", name=f"gshq_l{layer}_{m}"
                        )
                        nc.scalar.activation(
                            gs[:], pg[m][:], AF.Relu, scale=H_SCALE / OUT_SCALE
                        )
                        agin = dram.tile([128, SHARD], fp8, tag=f"agin{m}")
                        nc.gpsimd.dma_start(agin[:], gs[:])
                        agout = dram.tile(
                            [NCORES, 128, SHARD],
                            fp8,
                            tag=f"agout{m}",
                            addr_space="Shared",
                        )
                        nc.gpsimd.collective_compute(
                            "AllGather",
                            mybir.AluOpType.bypass,
                            replica_groups=rg,
                            ins=[agin.opt()],
                            outs=[agout.opt()],
                        )
                        nc.sync.dma_start(
                            g[:, m, :].rearrange("p (c j) -> p c j", c=NCORES),
                            agout[:, :, :].rearrange("c p j -> p c j"),
                        )
                else:
                    g3sh = []
                    for m in range(2):
                        gs = spool.tile(
                            [128, SHARD], bf16, tag=f"gsh{m}", name=f"gsh_l{layer}_{m}"
                        )
                        nc.scalar.activation(
                            gs[:], pg[m][:], AF.Relu, scale=H_SCALE / OUT_SCALE
                        )
                        g3sh.append(gs)

            # ---- fc1: z_partial[1, 256] per PE column group, 4 groups ----
            # M=1 matmuls run 4-wide across 32-col groups of the PE array;
            # partial sums land on PSUM partitions 0/32/64/96.
            # reuse a (dead) px PSUM buffer for the z partials: frees a
            # pzp bank so pxp can run 4 buffers during the layers. The 4
            # col-group chains touch disjoint partitions, so start=True's
            # bank-row zeroing stays safe.
            pz4 = pxp.tile([128, 512], f32, tag="px")
            for blk in range(FCK // FCB):
                if fc1_tiles[blk] is None:
                    issue_fc1()
                w16 = fc1_tiles[blk]
                for s in range(FCB):
                    t = blk * FCB + s
                    q = t % 4
                    nc.tensor.matmul(
                        pz4[32 * q : 32 * q + 1, 0:H],
                        lhsT=g3sh[t % 2][:, t // 2 : t // 2 + 1],
                        rhs=w16[:, s, :],
                        start=(t < 4),
                        stop=(t >= FCK - 4),
                        tile_position=(0, 32 * q),
                        skip_group_check=True,
                    )
                issue_fc1()  # keep the DMA stream ahead of consumption
            # copy only the 4 written partial rows (partition-preserving)
            zsb = spool.tile([128, H], f32, tag="zsb")
            for q in range(4):
                if q % 2 == 0:
                    nc.vector.tensor_copy(
                        zsb[32 * q : 32 * q + 1, :], pz4[32 * q : 32 * q + 1, 0:H]
                    )
                else:
                    nc.scalar.copy(
                        zsb[32 * q : 32 * q + 1, :], pz4[32 * q : 32 * q + 1, 0:H]
                    )

            # AllGather the 4 per-core z partials, then sum the 32 rows on
            # the PE with a K=32 ones-matmul — which also transposes z into
            # the [128, 1] column layout fc2 needs.
            agzin = dram.tile([4, H], f32, tag="agzin")
            nc.gpsimd.dma_start(
                agzin[:], zsb[:].rearrange("(a b) h -> a b h", b=32)[:, 0, :]
            )
            agzout = dram.tile([NCORES, 4, H], f32, tag="agzout", addr_space="Shared")
            nc.gpsimd.collective_compute(
                "AllGather",
                mybir.AluOpType.bypass,
                replica_groups=rg,
                ins=[agzin.opt()],
                outs=[agzout.opt()],
            )
            zparts = spool.tile([NCORES * 4, H], f32, tag="zparts")
            nc.sync.dma_start(zparts[:], agzout[:].rearrange("c a h -> (c a) h"))
            ones32 = wpool.tile([NCORES * 4, 1], f32)
            nc.gpsimd.memset(ones32[:], 1.0)

            # NOTE: matmul start=True zeroes the whole 2KB PSUM bank row
            # per touched partition (ZERO_REGION_SIZE), so independent
            # accumulators must live in separate banks.
            po = pzp.tile([C, 1], f32, tag="po")
            pozc = pzp.tile([128, 2], f32, tag="pozc")
            for k in range(2):
                nc.tensor.matmul(
                    pozc[:, k : k + 1],
                    lhsT=zparts[:, k * 128 : (k + 1) * 128],
                    rhs=ones32[:],
                    start=(k == 0),
                    stop=(k == 1),
                    skip_group_check=True,
                )
            for k in range(2):
                zr = spool.tile([128, 1], bf16, tag=f"zr{k}")
                nc.scalar.activation(
                    zr[:],
                    pozc[:, k : k + 1],
                    AF.Relu,
                    bias=fc1b_sb[:, k : k + 1],
                    scale=1.0 / (FC1_SCALE * H_SCALE),
                )
                nc.tensor.matmul(
                    po[:],
                    lhsT=fc2t_sb[:, k * C : (k + 1) * C],
                    rhs=zr[:],
                    start=(k == 0),
                    stop=(k == 1),
                )
            osb = spool.tile([C, 1], f32, tag="osb")
            nc.scalar.activation(osb[:], po[:], AF.Sigmoid, bias=fc2b_sb[:, 0:1])
            # sync queue: the gpsimd queue's tail drain is ~7us slower
            nc.sync.dma_start(OUT[:], osb[:])

    nc.compile()
    _split_drain_waits(nc)
    return nc


def _prep_shared(feat, W0, b0, W1, b1, W2, b2, fc1_b, fc2_w, fc2_b):
    """Host layout prep for the tensors every core receives identically."""
    g0 = (
        (np.ascontiguousarray(feat.T) * H_SCALE)
        .astype(FP8)
        .reshape(2, 128, N_OBJ)
        .transpose(1, 0, 2)
    )
    g0 = np.ascontiguousarray(g0)

    # wt[p, layer, k, e*H+h] = W_layer[e][h, k*128+p] * W_SCALE
    wt = np.empty((128, 3, 2, NE * H), dtype=FP8)
    for li, W in enumerate((W0, W1, W2)):
        for e in range(NE):
            wte = np.ascontiguousarray(W[e].T).astype(np.float32) * W_SCALE  # [F, H]
            wt[:, li, 0, e * H : (e + 1) * H] = wte[:128].astype(FP8)
            wt[:, li, 1, e * H : (e + 1) * H] = wte[128:].astype(FP8)

    # bia[e, (layer, m), p] = b_layer[e][m*128+p] * OUT_SCALE
    bia = np.empty((NE, 3 * 2, 128), dtype=BF16)
    for li, b in enumerate((b0, b1, b2)):
        sc = (np.asarray(b, dtype=np.float32) * OUT_SCALE).reshape(NE, 2, 128)
        bia[:, li * 2 : li * 2 + 2, :] = sc.astype(BF16)

    fc1b = np.ascontiguousarray(fc1_b.reshape(2, 128).T).astype(np.float32)
    fc2t = np.ascontiguousarray(
        fc2_w.T.reshape(2, 128, C).transpose(1, 0, 2).reshape(128, 2 * C)
    ).astype(BF16)
    fc2b = fc2_b.reshape(C, 1).astype(np.float32)
    return g0, wt, bia, fc1b, fc2t, fc2b


def _prep_graph(edges):
    """Per-(etype, core) normalized adjacency slices + degree indicators."""
    atp = np.empty((NCORES, NE, NP // ABLK, 128, ABLK * 2 * SHARD), dtype=FP8)
    ind = np.empty((NCORES, NE, SHARD), dtype=BF16)
    for e in range(NE):
        src = np.asarray(edges[e, 0], dtype=np.int64)
        dst = np.asarray(edges[e, 1], dtype=np.int64)
        deg = np.bincount(dst, minlength=N_OBJ).astype(np.float32)
        w = A_SCALE / np.maximum(deg, 1.0)
        an = np.zeros((N_OBJ, N_OBJ), dtype=np.float32)
        np.add.at(an, (src, dst), w[dst])
        an8 = an.astype(FP8)
        ind_e = (deg > 0).astype(BF16)
        for c in range(NCORES):
            sh = an8[:, c * SHARD : (c + 1) * SHARD]  # [4096 src, 512]
            # [NP//ABLK blocks, 128 part, ABLK pairs, 2 planes, 512]
            atp[c, e] = (
                sh.reshape(NP // ABLK, ABLK, 2, 128, SHARD)
                .transpose(0, 3, 1, 2, 4)
                .reshape(NP // ABLK, 128, ABLK * 2 * SHARD)
            )
            ind[c, e] = ind_e[c * SHARD : (c + 1) * SHARD]
    return atp, ind


def _prep_fc1(fc1_w):
    """Per-core column slice of fc1_w, transposed and DMA-batch packed."""
    out = []
    ksl = SHARD * H  # 131072 flat positions per core
    for c in range(NCORES):
        sl = np.ascontiguousarray(fc1_w[:, c * ksl : (c + 1) * ksl].T)  # [131072, 256]
        packed = (
            (sl.reshape(FCK // FCB, FCB, 128, H) * FC1_SCALE)
            .transpose(0, 2, 1, 3)
            .reshape(FCK // FCB, 128, FCB * H)
            .astype(FP8)
        )
        out.append(packed)
    return out


def kernel(feat, edges, W0, b0, W1, b1, W2, b2, fc1_w, fc1_b, fc2_w, fc2_b):
    from concourse.bass_utils import run_bass_kernel_spmd

    if "nc" not in _BASS_CACHE:
        _BASS_CACHE["nc"] = _build_bass()
    nc = _BASS_CACHE["nc"]

    in_maps = _make_in_maps(
        dict(
            feat=feat, edges=edges, W0=W0, b0=b0, W1=W1, b1=b1, W2=W2, b2=b2,
            fc1_w=fc1_w, fc1_b=fc1_b, fc2_w=fc2_w, fc2_b=fc2_b,
        )
    )

    res = run_bass_kernel_spmd(nc, in_maps, core_ids=list(range(NCORES)))
    out = np.asarray(res.results[0]["out"]).reshape(C)
    return out.astype(np.float32)


def _make_in_maps(inputs):
    g0, wt, bia, fc1b, fc2t, fc2b = _prep_shared(
        np.asarray(inputs["feat"], dtype=np.float32),
        np.asarray(inputs["W0"]), np.asarray(inputs["b0"]),
        np.asarray(inputs["W1"]), np.asarray(inputs["b1"]),
        np.asarray(inputs["W2"]), np.asarray(inputs["b2"]),
        np.asarray(inputs["fc1_b"]), np.asarray(inputs["fc2_w"]),
        np.asarray(inputs["fc2_b"]),
    )
    atp, ind = _prep_graph(np.asarray(inputs["edges"]))
    fc1t = _prep_fc1(np.asarray(inputs["fc1_w"]))
    return [
        {
            "g0": g0, "atp": atp[c], "wt": wt, "bia": bia, "ind": ind[c],
            "fc1t": fc1t[c], "fc1b": fc1b, "fc2t": fc2t, "fc2b": fc2b,
        }
        for c in range(NCORES)
    ]


def run_profiled(inputs, trace_cores=None):
    """Test-only: run with NTFF tracing; returns BassKernelResults."""
    from concourse import bass_utils
    from concourse.bass_utils import run_bass_kernel_spmd

    bass_utils.upload_artifacts = lambda tmpdir: f"local://{tmpdir}"
    if "nc" not in _BASS_CACHE:
        _BASS_CACHE["nc"] = _build_bass()
    nc = _BASS_CACHE["nc"]
    in_maps = _make_in_maps(inputs)
    tmpdir = "/tmp/gcn_profile"
    import shutil, os
    shutil.rmtree(tmpdir, ignore_errors=True)
    os.makedirs(tmpdir, exist_ok=True)
    return run_bass_kernel_spmd(
        nc,
        in_maps,
        core_ids=list(range(NCORES)),
        trace=True,
        tmpdir=tmpdir,
        trace_cores=trace_cores,
    )
